# revision 19
# baseline (speedup 1.0000x reference)
"""Trainium2 Bass kernel for GCN(+self-loops, sym-norm) + CBAM block. v3.

Per core (SPMD identical program; per-core differences via inputs only):
  1. Own-shard h' = (x@W)*dinv (bf16) from a host-transposed, even/odd
     column-permuted xT shard (no PE transposes); written pair-packed
     (256B stride fully used) and AllGathered -> 12.8MB full table.
  2. Edge aggregation: bulk dma_gather, 4 SWDGE queues, one per
     (half, parity) stream of AG-ordered pair positions; one-hot (bf16)
     scatter matmuls accumulate per-dst-block PSUM; dinv[dst] on the
     PSUM->SBUF copy. Self-loop terms and running channel sum/max stats
     are folded per-supergroup so they hide under SWDGE desc-gen.
     Dst blocks are rebalanced across cores (sorted-stripe assignment)
     to cut shared-capacity padding.
  3. Channel stats cross-partition reduce + tiny AllGather + CBAM MLP.
  4. Epilogue: bias, channel gate, spatial gate, residual, relu; output
     unpermuted via two stride-2-row DMAs; host reorders blocks.
"""

import sys

for _p in ("/opt/trn_rl_repo", "/root/.axon_site/_ro/trn_rl_repo"):
    if _p not in sys.path:
        sys.path.insert(0, _p)

from contextlib import ExitStack

import numpy as np
import ml_dtypes

import concourse.bass as bass
import concourse.tile as tile
from concourse import bacc, mybir
from concourse.bass import AP
from concourse.bass_utils import run_bass_kernel_spmd
from concourse.masks import make_identity

P = 128
F32 = mybir.dt.float32
BF16 = mybir.dt.bfloat16
I16 = mybir.dt.int16
AF = mybir.ActivationFunctionType
ALU = mybir.AluOpType

N_CORES = 8
N = 100000
C = 64
NB = 98
NSH = NB * P
NPAD = N_CORES * NSH
NPAIR = NPAD // 2
HALFP = NPAIR // 2
QR = NPAD // 4              # rows per gather quarter

PERM = np.concatenate([np.arange(0, P, 2), np.arange(1, P, 2)])
PERM_POS = np.empty(P, np.int64)
PERM_POS[PERM] = np.arange(P)


def _patch_dma_gather():
    """Relax the elem_size %256B assert for non-transpose gathers (the Q7
    ucode only requires the row STRIDE to be a multiple of 256B)."""
    import inspect
    import textwrap

    if getattr(bass.BassGpSimd.dma_gather, "_elem_patch", False):
        return
    src = inspect.getsource(bass.BassGpSimd.dma_gather)
    src = src.replace(
        "assert (\n            elem_size_bytes > 0 and elem_size_bytes"
        " % 256 == 0\n        )  # transpose restriction",
        "assert elem_size_bytes > 0 and"
        " (not transpose or elem_size_bytes % 256 == 0)")
    ns = dict(bass.BassGpSimd.dma_gather.__globals__)
    exec(textwrap.dedent(src), ns)
    ns["dma_gather"]._elem_patch = True
    bass.BassGpSimd.dma_gather = ns["dma_gather"]


_patch_dma_gather()


def preprocess(edge_index, n_unused=None, cores_unused=None, sg_max_tiles=96):
    src = np.asarray(edge_index[0], np.int64)
    dst = np.asarray(edge_index[1], np.int64)

    deg = np.bincount(dst, minlength=NPAD).astype(np.float32)
    deg[:N] += 1.0
    dinv = np.zeros(NPAD, np.float32)
    nz = deg > 0
    dinv[nz] = 1.0 / np.sqrt(deg[nz])

    NG = N_CORES * NB
    gblk = dst // P                           # global dst block of each edge
    # sorted-stripe rebalance: blocks with similar edge counts share a
    # local index across cores -> max-over-cores ~ mean -> less padding
    gtot = np.bincount(gblk, minlength=NG)
    order = np.argsort(-gtot, kind="stable")  # rank -> global block
    core_of = np.empty(NG, np.int64)
    local_of = np.empty(NG, np.int64)
    for r, g in enumerate(order):
        core_of[g] = r % N_CORES
        local_of[g] = r // N_CORES
    owned = np.empty((N_CORES, NB), np.int64)  # [core, lb] -> global block
    owned[core_of, local_of] = np.arange(NG)

    core = core_of[gblk]
    blk = local_of[gblk]
    dpos = PERM_POS[dst % P].astype(np.float32)

    # AG-ordered table position of src: owner shard base + permuted local row
    sg_ = src // P
    tpos = (core_of[sg_] * NSH + local_of[sg_] * P
            + PERM_POS[src % P])
    stream = tpos // QR
    idx16 = (tpos - stream * QR).astype(np.int16)

    key = (core * NB + blk) * 4 + stream
    order_e = np.argsort(key, kind="stable")
    key_s = key[order_e]
    idx_s = idx16[order_e]
    dpos_s = dpos[order_e]
    counts = np.bincount(key_s, minlength=N_CORES * NB * 4)
    grp_start = np.concatenate([[0], np.cumsum(counts)[:-1]])
    cnt = counts.reshape(N_CORES, NB, 4)

    cap = -(-cnt.max(axis=0) // P) * P
    blk_tiles = cap.sum(axis=1) // P

    sgs, cur, cur_t = [], [], 0
    for b in range(NB):
        t = int(blk_tiles[b])
        if cur and cur_t + t > sg_max_tiles:
            sgs.append(cur)
            cur, cur_t = [], 0
        cur.append(b)
        cur_t += t
    if cur:
        sgs.append(cur)

    total_tiles = int(blk_tiles.sum())
    TOT = total_tiles * P

    sg_meta = []
    pos_ = 0
    seg_pos = np.zeros((NB, 4), np.int64)
    for sg in sgs:
        q_ops = []
        blk_tilecols = {b: [] for b in sg}
        for q in range(4):
            q_start = pos_
            for b in sg:
                n_slots = int(cap[b, q])
                if n_slots == 0:
                    continue
                seg_pos[b, q] = pos_
                blk_tilecols[b].extend(range(pos_ // P, (pos_ + n_slots) // P))
                pos_ += n_slots
            q_ops.append((q, pos_ - q_start, q_start))
        sg_meta.append(dict(q_ops=q_ops,
                            blocks=[(b, blk_tilecols[b]) for b in sg]))
    assert pos_ == TOT
    # per-queue packed column base: queue q's idx data lives only at
    # partition rows 32q..32q+31 (the two Q7 cores that serve queue q),
    # so all four queues share the same columns at different row stripes.
    qcur = [0, 0, 0, 0]
    for sgm in sg_meta:
        q_ops2 = []
        for q, nq, s0 in sgm["q_ops"]:
            q_ops2.append((q, nq, s0, qcur[q]))
            qcur[q] += nq
        sgm["q_ops"] = q_ops2
    icols = max(qcur) // 16

    idx_wraps, dstl_arrs = [], []
    for c in range(N_CORES):
        idx_flat = np.zeros(TOT, np.int16)
        dstl_flat = np.full(TOT, -1.0, np.float32)
        for b in range(NB):
            for q in range(4):
                n_edges = int(cnt[c, b, q])
                if n_edges == 0:
                    continue
                g0 = grp_start[(c * NB + b) * 4 + q]
                s0 = seg_pos[b, q]
                idx_flat[s0:s0 + n_edges] = idx_s[g0:g0 + n_edges]
                dstl_flat[s0:s0 + n_edges] = dpos_s[g0:g0 + n_edges]
        iw = np.zeros((P, icols), np.int16)
        for q in range(4):
            segs = [idx_flat[s0:s0 + nq]
                    for sgm in sg_meta
                    for qq, nq, s0, _ in sgm["q_ops"] if qq == q and nq]
            own = np.concatenate(segs) if segs else np.zeros(0, np.int16)
            wq = own.reshape(-1, 16).T
            iw[32 * q:32 * q + 16, :wq.shape[1]] = wq
            iw[32 * q + 16:32 * q + 32, :wq.shape[1]] = wq
        idx_wraps.append(iw)
        dstl_arrs.append(
            dstl_flat.reshape(total_tiles, P).T.astype(ml_dtypes.bfloat16))

    # perm-space dinv per (pos, global block)
    node_pg = np.arange(NG)[None, :] * P + PERM[:, None]
    dinva = dinv[node_pg].astype(np.float32)          # [P, NG]

    return dict(total_tiles=total_tiles, sg_meta=sg_meta, icols=icols,
                idx_wraps=idx_wraps, dstl_arrs=dstl_arrs,
                dinv=dinv, dinva=dinva, owned=owned,
                core_of=core_of, local_of=local_of)


def mid_bcast(ap2d: AP, n: int) -> AP:
    (pstep, pcnt), (istep, icnt) = ap2d.ap
    return AP(ap2d.tensor, ap2d.offset, [[pstep, pcnt], [0, n], [istep, icnt]])


def view3(ap2d: AP, d1: int, d2: int, transpose=False) -> AP:
    (pstep, pcnt), (istep, icnt) = ap2d.ap
    assert icnt == d1 * d2
    if transpose:
        return AP(ap2d.tensor, ap2d.offset,
                  [[pstep, pcnt], [istep, d2], [istep * d2, d1]])
    return AP(ap2d.tensor, ap2d.offset,
              [[pstep, pcnt], [istep * d2, d1], [istep, d2]])


def build_nc(meta, debug: bool = False):
    TT = meta["total_tiles"]
    H = 4  # C // 16

    nc = bacc.Bacc("TRN2", target_bir_lowering=False, debug=debug,
                   enable_asserts=True, num_devices=N_CORES,
                   num_swdge_queues=4)

    xTs = nc.dram_tensor("xTs", [C, NSH], BF16, kind="ExternalInput")
    IC = meta["icols"]
    idxw = nc.dram_tensor("idxw", [P, IC], I16, kind="ExternalInput")
    dstl = nc.dram_tensor("dstl", [P, TT], BF16, kind="ExternalInput")
    dinvd = nc.dram_tensor("dinvd", [P, NB], F32, kind="ExternalInput")
    Wbf = nc.dram_tensor("Wbf", [C, C], BF16, kind="ExternalInput")
    brow = nc.dram_tensor("brow", [1, C], F32, kind="ExternalInput")
    w1 = nc.dram_tensor("w1", [C, H], F32, kind="ExternalInput")
    w2 = nc.dram_tensor("w2", [H, C], F32, kind="ExternalInput")
    sprow = nc.dram_tensor("sprow", [1, 3], F32, kind="ExternalInput")
    out = nc.dram_tensor("out", [NSH, C], F32, kind="ExternalOutput")

    hsh = nc.dram_tensor("hsh", [NSH, 2 * C], BF16)
    hfull = nc.dram_tensor("hfull", [NPAD, 2 * C], BF16, addr_space="Shared")
    stats_loc = nc.dram_tensor("stats_loc", [P, 1], F32)
    stats_ag = nc.dram_tensor("stats_ag", [P * N_CORES, 1], F32,
                              addr_space="Shared")
    rg = [list(range(N_CORES))]

    with tile.TileContext(nc) as tc, ExitStack() as ctx:
        const = ctx.enter_context(tc.tile_pool(name="const", bufs=1))
        big = ctx.enter_context(tc.tile_pool(name="big", bufs=1))
        pmisc = ctx.enter_context(tc.tile_pool(name="pmisc", bufs=2,
                                               space="PSUM"))

        # ---- constants ----
        ident = const.tile([P, P], F32)
        make_identity(nc, ident[:])
        iota_i = const.tile([P, P], mybir.dt.int32)
        nc.gpsimd.iota(iota_i[:], pattern=[[1, P]], channel_multiplier=0)
        iota_b = const.tile([P, P], BF16)
        nc.vector.tensor_copy(iota_b[:], iota_i[:])
        ones_row = const.tile([1, P], F32)
        nc.gpsimd.memset(ones_row[:], 1.0)
        ones2 = const.tile([2, 1], F32)
        nc.gpsimd.memset(ones2[:], 1.0)

        W_sb = const.tile([C, C], BF16)
        nc.sync.dma_start(W_sb[:], Wbf.ap())
        brow_sb = const.tile([1, C], F32)
        nc.sync.dma_start(brow_sb[:], brow.ap())
        w1_sb = const.tile([C, H], F32)
        nc.sync.dma_start(w1_sb[:], w1.ap())
        w2_sb = const.tile([H, C], F32)
        nc.sync.dma_start(w2_sb[:], w2.ap())
        sprow_sb = const.tile([1, 3], F32)
        nc.sync.dma_start(sprow_sb[:], sprow.ap())
        dinvd_sb = const.tile([P, NB], F32)
        nc.sync.dma_start(dinvd_sb[:], dinvd.ap())

        bb_ps = pmisc.tile([P, C], F32, space="PSUM", tag="mm")
        nc.tensor.matmul(bb_ps[:], lhsT=ones_row[:], rhs=brow_sb[:],
                         start=True, stop=True)
        b_bc = const.tile([P, C], F32)
        nc.scalar.copy(b_bc[:], bb_ps[:])
        sp_ps = pmisc.tile([P, 3], F32, space="PSUM", tag="mm")
        nc.tensor.matmul(sp_ps[:], lhsT=ones_row[:], rhs=sprow_sb[:],
                         start=True, stop=True)
        sp_bc = const.tile([P, 3], F32)
        nc.scalar.copy(sp_bc[:], sp_ps[:])
        bt_ps = pmisc.tile([C, 1], F32, space="PSUM", tag="mm")
        nc.tensor.transpose(bt_ps[:], in_=brow_sb[:], identity=ident[:1, :1])
        bT = const.tile([C, 1], F32)
        nc.scalar.copy(bT[:], bt_ps[:])

        # ---- persistent phase-2 state ----
        idxw_sb = big.tile([P, IC], I16)
        nc.sync.dma_start(idxw_sb[:], idxw.ap())
        dstl_sb = big.tile([P, TT], BF16)
        nc.sync.dma_start(dstl_sb[:], dstl.ap())
        agg_sb = big.tile([P, NB * C], F32)
        hb_own = big.tile([P, NB * C], BF16)
        stats2 = big.tile([P, 2 * C], F32)   # [sum | max] running stats
        nc.gpsimd.memset(stats2[:, 0:C], 0.0)
        nc.gpsimd.memset(stats2[:, C:2 * C], -1e30)

        # ---- phase 1: own-shard h', pair-packed, AllGather ----
        NGB = 13
        with tc.tile_pool(name="xc", bufs=1) as xcp, \
             tc.tile_pool(name="pp1", bufs=2, space="PSUM") as pp1:
            xts_sb = xcp.tile([C, NSH], BF16, tag="xts")
            nc.sync.dma_start(xts_sb[:], xTs.ap())
            for jb in range(NGB):
                nb8 = min(8, NB - jb * 8)
                bank = pp1.tile([P, nb8 * C], F32, space="PSUM", tag="bank")
                for k in range(nb8):
                    b = jb * 8 + k
                    nc.tensor.matmul(bank[:, k * C:(k + 1) * C],
                                     lhsT=xts_sb[:, b * P:(b + 1) * P],
                                     rhs=W_sb[:], start=True, stop=True)
                nc.vector.tensor_tensor(
                    out=view3(hb_own[:, jb * 8 * C:(jb * 8 + nb8) * C],
                              nb8, C),
                    in0=view3(bank[:], nb8, C),
                    in1=dinvd_sb[:, jb * 8:jb * 8 + nb8]
                        .to_broadcast([P, nb8, C]),
                    op=ALU.mult)
                nc.sync.dma_start(
                    AP(hsh, jb * 8 * P * 2 * C,
                       [[2 * C, P], [P * 2 * C, nb8], [1, C]]),
                    view3(hb_own[:, jb * 8 * C:(jb * 8 + nb8) * C],
                          nb8, C))
        nc.gpsimd.collective_compute(
            "AllGather", ALU.bypass, replica_groups=rg,
            ins=[hsh.ap()], outs=[hfull.ap()])

        # ---- phase 2: aggregation + hidden self-loop + running stats ----
        qtab = [AP(hfull, q * QR * 2 * C,
                   [[2 * C, QR], [1, C]]) for q in range(4)]

        def consume_sg(sg, tb, gath, onehot):
            """Matmuls + scaled copies + self-loop + running stats for one
            supergroup (emitted one sg behind the gather/IS_EQ front so the
            in-order DVE queue never stalls the onehot pipeline)."""
            for b, tcols in sg["blocks"]:
                if not tcols:
                    nc.gpsimd.memset(agg_sb[:, b * C:(b + 1) * C], 0.0)
                    continue
                agg_ps = pp2.tile([P, C], F32, space="PSUM", tag="agg")
                for j, t in enumerate(tcols):
                    nc.tensor.matmul(
                        agg_ps[:],
                        lhsT=onehot[:, (t - tb) * P:(t - tb + 1) * P],
                        rhs=gath[:, (t - tb) * C:(t - tb + 1) * C],
                        start=(j == 0), stop=(j == len(tcols) - 1))
                nc.scalar.activation(
                    out=agg_sb[:, b * C:(b + 1) * C], in_=agg_ps[:],
                    func=AF.Copy, scale=dinvd_sb[:, b:b + 1])
            b0, b1 = sg["blocks"][0][0], sg["blocks"][-1][0] + 1
            nbs = b1 - b0
            sl = slp.tile([P, nbs * C], F32, tag="sl")
            nc.vector.tensor_tensor(
                out=view3(sl[:], nbs, C),
                in0=view3(hb_own[:, b0 * C:b1 * C], nbs, C),
                in1=dinvd_sb[:, b0:b1].to_broadcast([P, nbs, C]),
                op=ALU.mult)
            nc.vector.tensor_tensor(
                out=view3(agg_sb[:, b0 * C:b1 * C], nbs, C),
                in0=view3(agg_sb[:, b0 * C:b1 * C], nbs, C),
                in1=view3(sl[:], nbs, C), op=ALU.add)
            for b in range(b0, b1):
                nc.vector.tensor_tensor(
                    out=stats2[:, 0:C], in0=stats2[:, 0:C],
                    in1=agg_sb[:, b * C:(b + 1) * C], op=ALU.add)
                nc.vector.tensor_tensor(
                    out=stats2[:, C:2 * C], in0=stats2[:, C:2 * C],
                    in1=agg_sb[:, b * C:(b + 1) * C], op=ALU.max)

        with tc.tile_pool(name="gt", bufs=4) as gtp, \
             tc.tile_pool(name="oh", bufs=4) as ohp, \
             tc.tile_pool(name="slp", bufs=2) as slp, \
             tc.tile_pool(name="pp2", bufs=2, space="PSUM") as pp2:
            pending = None
            for sgi, sg in enumerate(meta["sg_meta"]):
                tb = sg["q_ops"][0][2] // P
                ntile = sum(nq for _, nq, _, _ in sg["q_ops"]) // P
                gath = gtp.tile([P, ntile * C], BF16, tag="gath")
                for q, nq, s0, qp in sg["q_ops"]:
                    if nq == 0:
                        continue
                    col = (s0 // P - tb) * C
                    nc.gpsimd.dma_gather(
                        out_ap=view3(gath[:, col:col + (nq // P) * C],
                                     nq // P, C),
                        in_ap=qtab[q],
                        idxs_ap=idxw_sb[:, qp // 16:(qp + nq) // 16],
                        num_idxs=nq, num_idxs_reg=nq,
                        elem_size=C, elem_step=2 * C,
                        single_packet=False, queue_num=q)
                onehot = ohp.tile([P, ntile * P], BF16, tag="oh")
                nc.vector.tensor_tensor(
                    out=view3(onehot[:], ntile, P),
                    in0=dstl_sb[:, tb:tb + ntile].to_broadcast([P, ntile, P]),
                    in1=mid_bcast(iota_b[:], ntile),
                    op=ALU.is_equal)
                if pending is not None:
                    consume_sg(*pending)
                pending = (sg, tb, gath, onehot)
            consume_sg(*pending)

        # ---- phase 3: cross-partition stats + AllGather + CBAM MLP ----
        sT_ps = pmisc.tile([2 * C, P], F32, space="PSUM", tag="mm")
        nc.tensor.transpose(sT_ps[:], in_=stats2[:], identity=ident[:])
        sT_sb = const.tile([2 * C, P], F32)
        nc.scalar.copy(sT_sb[:], sT_ps[:])
        loc = const.tile([P, 1], F32)
        nc.vector.reduce_sum(loc[0:C, :], sT_sb[0:C, :],
                             axis=mybir.AxisListType.X)
        nc.vector.reduce_max(loc[C:2 * C, :], sT_sb[C:2 * C, :],
                             axis=mybir.AxisListType.X)
        nc.sync.dma_start(stats_loc.ap(), loc[:])
        nc.gpsimd.collective_compute(
            "AllGather", ALU.bypass, replica_groups=rg,
            ins=[stats_loc.ap()], outs=[stats_ag.ap()])
        ag_sb = const.tile([P, N_CORES], F32)
        nc.sync.dma_start(ag_sb[:], AP(stats_ag, 0, [[1, P], [P, N_CORES]]))
        gsum = const.tile([C, 1], F32)
        nc.vector.reduce_sum(gsum[:], ag_sb[0:C, :], axis=mybir.AxisListType.X)
        gmax_hi = const.tile([P, 1], F32)
        nc.vector.reduce_max(gmax_hi[C:2 * C, :], ag_sb[C:2 * C, :],
                             axis=mybir.AxisListType.X)
        gmax = const.tile([C, 1], F32)
        nc.sync.dma_start(gmax[:], gmax_hi[C:2 * C, :])

        v2 = const.tile([C, 2], F32)
        nc.vector.tensor_scalar(out=v2[:, 0:1], in0=gsum[:], scalar1=1.0 / N,
                                scalar2=bT[:], op0=ALU.mult, op1=ALU.add)
        nc.vector.tensor_scalar(out=v2[:, 1:2], in0=gmax[:], scalar1=bT[:],
                                scalar2=None, op0=ALU.add)

        r1_ps = pmisc.tile([2, H], F32, space="PSUM", tag="mm")
        nc.tensor.matmul(r1_ps[:], lhsT=v2[:], rhs=w1_sb[:], start=True,
                         stop=True)
        r1_sb = const.tile([2, H], F32)
        nc.scalar.activation(r1_sb[:], r1_ps[:], func=AF.Relu)
        r1T_ps = pmisc.tile([H, 2], F32, space="PSUM", tag="mm")
        nc.tensor.transpose(r1T_ps[:], in_=r1_sb[:], identity=ident[:2, :2])
        r1T_sb = const.tile([H, 2], F32)
        nc.scalar.copy(r1T_sb[:], r1T_ps[:])
        r2_ps = pmisc.tile([2, C], F32, space="PSUM", tag="mm")
        nc.tensor.matmul(r2_ps[:], lhsT=r1T_sb[:], rhs=w2_sb[:], start=True,
                         stop=True)
        r2_sb = const.tile([2, C], F32)
        nc.scalar.copy(r2_sb[:], r2_ps[:])
        cal_ps = pmisc.tile([1, C], F32, space="PSUM", tag="mm")
        nc.tensor.matmul(cal_ps[:], lhsT=ones2[:], rhs=r2_sb[:], start=True,
                         stop=True)
        ca_sb = const.tile([1, C], F32)
        nc.scalar.activation(ca_sb[:], cal_ps[:], func=AF.Sigmoid)
        cab_ps = pmisc.tile([P, C], F32, space="PSUM", tag="mm")
        nc.tensor.matmul(cab_ps[:], lhsT=ones_row[:], rhs=ca_sb[:],
                         start=True, stop=True)
        cab = const.tile([P, C], F32)
        nc.scalar.copy(cab[:], cab_ps[:])

        # ---- phase 4: epilogue (2 chunks so the out-DMA overlaps) ----
        ph4 = ctx.enter_context(tc.tile_pool(name="ph4", bufs=1))
        ab_sb = ph4.tile([P, NB * C], F32)
        hg_sb = ph4.tile([P, NB * C], F32)
        for c0, c1 in ((0, 49), (49, NB)):
            nb_ = c1 - c0
            ab = ab_sb[:, c0 * C:c1 * C]
            hg = hg_sb[:, c0 * C:c1 * C]
            agg = agg_sb[:, c0 * C:c1 * C]
            nc.vector.tensor_tensor(out=view3(ab, nb_, C),
                                    in0=view3(agg, nb_, C),
                                    in1=mid_bcast(b_bc[:], nb_), op=ALU.add)
            nc.vector.tensor_tensor(out=view3(hg, nb_, C),
                                    in0=view3(ab, nb_, C),
                                    in1=mid_bcast(cab[:], nb_), op=ALU.mult)
            rsum = const.tile([P, nb_], F32, tag=f"rs{c0}")
            nc.vector.reduce_sum(rsum[:], view3(hg, nb_, C),
                                 axis=mybir.AxisListType.X)
            rmax = const.tile([P, nb_], F32, tag=f"rm{c0}")
            nc.vector.reduce_max(rmax[:], view3(hg, nb_, C),
                                 axis=mybir.AxisListType.X)
            t1 = const.tile([P, nb_], F32, tag=f"t1{c0}")
            nc.vector.tensor_scalar(out=t1[:], in0=rsum[:],
                                    scalar1=sp_bc[:, 0:1],
                                    scalar2=None, op0=ALU.mult)
            nc.vector.tensor_scalar(out=rmax[:], in0=rmax[:],
                                    scalar1=sp_bc[:, 1:2],
                                    scalar2=None, op0=ALU.mult)
            nc.vector.tensor_tensor(out=t1[:], in0=t1[:], in1=rmax[:],
                                    op=ALU.add)
            sa = const.tile([P, nb_], F32, tag=f"sa{c0}")
            nc.scalar.activation(sa[:], t1[:], func=AF.Sigmoid,
                                 bias=sp_bc[:, 2:3], scale=1.0)
            nc.vector.tensor_tensor(out=view3(hg, nb_, C),
                                    in0=view3(hg, nb_, C),
                                    in1=sa[:].to_broadcast([P, nb_, C]),
                                    op=ALU.mult)
            nc.vector.tensor_tensor(out=view3(ab, nb_, C),
                                    in0=view3(ab, nb_, C),
                                    in1=view3(hg, nb_, C), op=ALU.add)
            nc.scalar.activation(ab, ab, func=AF.Relu)
            nc.sync.dma_start(
                AP(out, c0 * P * C, [[2 * C, 64], [P * C, nb_], [1, C]]),
                view3(ab_sb[0:64, c0 * C:c1 * C], nb_, C))
            nc.sync.dma_start(
                AP(out, c0 * P * C + C,
                   [[2 * C, 64], [P * C, nb_], [1, C]]),
                view3(ab_sb[64:128, c0 * C:c1 * C], nb_, C))

    nc.compile()
    return nc


def make_xT_bf(x):
    """Full padded, even/odd column-permuted xT in bf16 [C, NPAD]."""
    x_pad = np.zeros((NPAD, C), np.float32)
    x_pad[:N] = np.asarray(x, np.float32)
    xp = x_pad.reshape(N_CORES * NB, P, C)[:, PERM, :]
    return np.ascontiguousarray(
        xp.reshape(NPAD, C).T.astype(ml_dtypes.bfloat16))


def make_in_maps(inputs, pp, n_unused=None):
    sw = np.asarray(inputs["spatial_w"], np.float32)
    sb = np.asarray(inputs["spatial_b"], np.float32)
    sprow = np.array([[sw[0, 0] / C, sw[1, 0], sb[0]]], np.float32)
    xT_bf = make_xT_bf(inputs["x"])
    Wbf = np.asarray(inputs["W"], np.float32).astype(ml_dtypes.bfloat16)
    dinva = pp["dinva"]
    owned = pp["owned"]

    in_maps = []
    for c in range(N_CORES):
        og = owned[c]                      # [NB] global block ids
        cols = (og[:, None] * P + np.arange(P)[None, :]).reshape(-1)
        in_maps.append({
            "xTs": np.ascontiguousarray(xT_bf[:, cols]),
            "idxw": np.ascontiguousarray(pp["idx_wraps"][c]),
            "dstl": np.ascontiguousarray(pp["dstl_arrs"][c]),
            "dinvd": np.ascontiguousarray(dinva[:, og]),
            "Wbf": Wbf,
            "brow": np.asarray(inputs["b"], np.float32).reshape(1, C),
            "w1": np.asarray(inputs["mlp_w1"], np.float32),
            "w2": np.asarray(inputs["mlp_w2"], np.float32),
            "sprow": sprow,
        })
    return in_maps


_CACHE = {}


def kernel(x, edge_index, W, b, mlp_w1, mlp_w2, spatial_w, spatial_b):
    inputs = {
        "x": np.asarray(x, np.float32),
        "edge_index": np.asarray(edge_index),
        "W": np.asarray(W, np.float32),
        "b": np.asarray(b, np.float32),
        "mlp_w1": np.asarray(mlp_w1, np.float32),
        "mlp_w2": np.asarray(mlp_w2, np.float32),
        "spatial_w": np.asarray(spatial_w, np.float32),
        "spatial_b": np.asarray(spatial_b, np.float32),
    }
    pp = preprocess(inputs["edge_index"])
    key = ("v12", pp["total_tiles"])
    if key not in _CACHE:
        _CACHE[key] = (build_nc(pp), NSH)
    nc, _ = _CACHE[key]
    in_maps = make_in_maps(inputs, pp)
    res = run_bass_kernel_spmd(nc, in_maps, list(range(N_CORES)))
    # un-permute: core c's output rows are its owned global blocks
    out_full = np.empty((NPAD, C), np.float32)
    for c in range(N_CORES):
        o = np.asarray(res.results[c]["out"], np.float32).reshape(NB, P, C)
        out_full.reshape(N_CORES * NB, P, C)[pp["owned"][c]] = o
    return np.ascontiguousarray(out_full[:N])
